# Initial kernel scaffold
#
"""Causal multi-head attention (B=4, T=2048, D=1024, H=16, HD=64) on 8
Trainium2 NeuronCores.

Sharding: data-parallel over batch (4) x tensor-parallel over heads (2
groups of 8). Each core runs the same Bass program on its own input
slices; the host sums the two tensor-parallel partial projections per
batch and adds b_proj.

Per-core dataflow (feature-major, no on-chip transposes):
  xT [D,T] (host pre-transposed)
  Q^T,K^T = w_{q,k}-stationary matmuls -> [512, T] feature-major
  V       = xT-stationary matmuls      -> [T, 512] token-major (+ones col)
  S^T     = K^T-block-stationary matmuls, 2 heads row-packed in the
            128-deep PE array (contraction = hd = 64)
  P       = exp(S^T) on ScalarE (1/8 scale folded into wq on host),
            causal handled by partial-N matmuls + affine_select on
            diagonal blocks
  O^T,den = V|1-stationary matmul accumulating over tk blocks (the ones
            column yields the softmax denominator in PSUM row 64)
  out     = O^T * (1/den) via reciprocal_approx + DRAM-bounce broadcast
  yT      = w_proj-stationary matmuls -> [D, T] partial (host reduces)

All matmul operands are float32r (streams at 1 cycle/row on the TRN2 PE
vs 4 for float32; ~1.3e-4 relative precision, measured).
"""

import numpy as np

import concourse.bass as bass
import concourse.bacc as bacc
import concourse.mybir as mybir
import concourse.tile as tile
from concourse.bass_utils import run_bass_kernel_spmd

F32 = mybir.dt.float32
F32R = mybir.dt.float32r
AF = mybir.ActivationFunctionType

B, T, D = 4, 2048, 1024
H, HD = 16, 64
NH = 8          # heads per core
DL = NH * HD    # 512 local qkv feature dim
PAIRS = NH // 2
CH = T // 512   # 4 chunks of 512 tokens
KT = T // 128   # 16 tk blocks / token tiles
VW = 65         # V columns per head incl. ones column

USE_F32R = True
MMDT = F32R if USE_F32R else F32


def build(nc: bass.Bass):
    xT = nc.declare_dram_parameter("xT", [D, T], MMDT, isOutput=False)
    wq = nc.declare_dram_parameter("wq", [D, DL], MMDT, isOutput=False)
    wk = nc.declare_dram_parameter("wk", [D, DL], MMDT, isOutput=False)
    wv = nc.declare_dram_parameter("wv", [D, DL], MMDT, isOutput=False)
    bq = nc.declare_dram_parameter("bq", [DL], F32, isOutput=False)
    bk = nc.declare_dram_parameter("bk", [DL], F32, isOutput=False)
    bv = nc.declare_dram_parameter("bv", [DL], F32, isOutput=False)
    wp = nc.declare_dram_parameter("wp", [DL, D], MMDT, isOutput=False)
    ones8 = nc.declare_dram_parameter("ones8", [128, 40], MMDT, isOutput=False)
    yT = nc.declare_dram_parameter("yT", [D, T], F32, isOutput=True)

    with tile.TileContext(nc) as tc:
        with (
            tc.tile_pool(name="persist", bufs=1) as persist,
            tc.tile_pool(name="dram", bufs=4, space="DRAM") as dram,
        ):
            # -------- persistent tiles --------
            qkT = [persist.tile([128, T], MMDT, tag=f"qk{i}", name=f"qk{i}")
                   for i in range(8)]
            v_sb = [persist.tile([128, NH * VW], MMDT, tag=f"v{i}",
                                 name=f"v{i}") for i in range(KT)]
            osb = [persist.tile([128, T], MMDT, tag=f"o{i}", name=f"o{i}")
                   for i in range(4)]
            bq_sb = persist.tile([128, 4], F32, tag="bq", name="bq_sb")
            bk_sb = persist.tile([128, 4], F32, tag="bk", name="bk_sb")
            bv_bc = persist.tile([128, DL], F32, tag="bv", name="bv_bc")
            nc.sync.dma_start(
                out=bq_sb, in_=bq[:].rearrange("(a p) -> p a", p=128)
            )
            nc.sync.dma_start(
                out=bk_sb, in_=bk[:].rearrange("(a p) -> p a", p=128)
            )
            nc.sync.dma_start(
                out=bv_bc,
                in_=bass.AP(tensor=bv[:].tensor, offset=0,
                            ap=[[0, 128], [1, DL]]),
            )

            # ================ phase 1: QKV projections ================
            with (
                nc.named_scope("qkv"),
                tc.tile_pool(name="ph1sb", bufs=1) as ph1sb,
                tc.tile_pool(name="ph1ps", bufs=6, space="PSUM") as ph1ps,
            ):
                wv_sb = [ph1sb.tile([128, DL], MMDT, tag=f"wv{k}",
                                    name=f"wv{k}") for k in range(8)]
                for k in range(8):
                    nc.sync.dma_start(
                        out=wv_sb[k], in_=wv[128 * k : 128 * k + 128, :]
                    )
                for c in range(CH):
                    cs = slice(512 * c, 512 * c + 512)
                    xt = []
                    for k in range(8):
                        t_ = ph1sb.tile([128, 512], MMDT, tag="xt", bufs=12,
                                        name="xt")
                        nc.sync.dma_start(
                            out=t_, in_=xT[128 * k : 128 * k + 128, cs]
                        )
                        xt.append(t_)
                    # V token-major (first: attention waits on all of V)
                    for t4 in range(4):
                        tt = 4 * c + t4
                        acc = ph1ps.tile([128, 512], F32, tag="ps", name="acc")
                        for k in range(8):
                            nc.tensor.matmul(
                                acc,
                                xt[k][:, 128 * t4 : 128 * t4 + 128],
                                wv_sb[k],
                                start=(k == 0),
                                stop=(k == 7),
                            )
                        # ones in col 64 of each 65-wide head block
                        nc.sync.dma_start(
                            out=v_sb[tt].rearrange("p (h c) -> p h c", c=VW)
                            [:, :, HD],
                            in_=ones8[:, 0:NH],
                        )
                        for h in range(NH):
                            nc.vector.tensor_add(
                                v_sb[tt][:, VW * h : VW * h + HD],
                                acc[:, HD * h : HD * h + HD],
                                bv_bc[:, HD * h : HD * h + HD],
                            )
                    # Q^T then K^T feature-major
                    for w_in, b_sb, obase in ((wq, bq_sb, 0), (wk, bk_sb, 4)):
                        wts = []
                        for k in range(8):
                            wt = ph1sb.tile([128, DL], MMDT, tag="w",
                                            bufs=8, name="wt")
                            nc.sync.dma_start(
                                out=wt, in_=w_in[128 * k : 128 * k + 128, :]
                            )
                            wts.append(wt)
                        for n in range(4):
                            acc = ph1ps.tile([128, 512], F32, tag="ps",
                                             name="acc")
                            for k in range(8):
                                nc.tensor.matmul(
                                    acc,
                                    wts[k][:, 128 * n : 128 * n + 128],
                                    xt[k],
                                    start=(k == 0), stop=(k == 7),
                                )
                            nc.vector.tensor_scalar_add(
                                out=qkT[obase + n][:, cs],
                                in0=acc,
                                scalar1=b_sb[:, n : n + 1],
                            )

            # prefetch proj weights (overlaps attention)
            at2sb_cm = tc.tile_pool(name="at2sb", bufs=1)
            at2sb = at2sb_cm.__enter__()
            wp_sb = [at2sb.tile([128, D], MMDT, tag=f"wp{k}",
                                name=f"wp{k}") for k in range(4)]
            for k in range(4):
                nc.sync.dma_start(
                    out=wp_sb[k], in_=wp[128 * k : 128 * k + 128, :]
                )

            # ================ phase 2: attention ================
            with (
                nc.named_scope("attn"),
                tc.tile_pool(name="atps", bufs=1, space="PSUM") as atps,
            ):
                for g2 in range(PAIRS):
                    qt, kt = qkT[g2], qkT[4 + g2]
                    ha, hb = 2 * g2, 2 * g2 + 1
                    for c in range(CH):
                        qs = slice(512 * c, 512 * c + 512)
                        av_a = atps.tile([VW, 512], F32, tag="av", bufs=4,
                                         name="av_a")
                        av_b = atps.tile([VW, 512], F32, tag="av", bufs=4,
                                         name="av_b")
                        nb = 4 * (c + 1)
                        for b in range(nb):
                            diag = (b // 4 == c)
                            off = 128 * (b - 4 * c) if diag else 0
                            bs = slice(128 * b, 128 * b + 128)
                            strip = atps.tile([128, 1024], F32, tag="strip",
                                              bufs=2, name="strip")
                            et = at2sb.tile([128, 1024], MMDT, tag="exp",
                                            bufs=4, name="et")
                            nc.tensor.matmul(
                                strip[:, off:512],
                                kt[0:64, bs],
                                qt[0:64, 512 * c + off : 512 * c + 512],
                                start=True, stop=True,
                            )
                            nc.tensor.matmul(
                                strip[:, 512 + off : 1024],
                                kt[64:128, bs],
                                qt[64:128, 512 * c + off : 512 * c + 512],
                                start=True, stop=True,
                            )
                            if off == 0:
                                nc.scalar.activation(
                                    et[:, 0:1024], strip[:, 0:1024], AF.Exp
                                )
                            else:
                                # one instr over both heads' valid regions:
                                # cols [off,512) and [512+off,1024)
                                w_ = 512 - off
                                src_ap = bass.AP(
                                    tensor=strip.tensor,
                                    offset=strip.offset + off,
                                    ap=[list(strip.ap[0]), [512, 2], [1, w_]],
                                )
                                dst_ap = bass.AP(
                                    tensor=et.tensor,
                                    offset=et.offset + off,
                                    ap=[list(et.ap[0]), [512, 2], [1, w_]],
                                )
                                nc.scalar.activation(dst_ap, src_ap, AF.Exp)
                            if diag:
                                for bcol in (off, 512 + off):
                                    nc.gpsimd.affine_select(
                                        out=et[:, bcol : bcol + 128],
                                        in_=et[:, bcol : bcol + 128],
                                        compare_op=mybir.AluOpType.is_ge,
                                        fill=0.0,
                                        base=0,
                                        pattern=[[1, 128]],
                                        channel_multiplier=-1,
                                    )
                            nc.tensor.matmul(
                                av_a[:, off:512],
                                v_sb[b][:, VW * ha : VW * ha + VW],
                                et[:, off:512],
                                start=(b == 0), stop=(b == nb - 1),
                            )
                            nc.tensor.matmul(
                                av_b[:, off:512],
                                v_sb[b][:, VW * hb : VW * hb + VW],
                                et[:, 512 + off : 1024],
                                start=(b == 0), stop=(b == nb - 1),
                            )
                        for h, av in ((0, av_a), (1, av_b)):
                            rec = at2sb.tile([1, 512], F32, tag="rec", bufs=4,
                                             name="rec")
                            scr = at2sb.tile([1, 512], F32, tag="scr", bufs=4,
                                             name="scr")
                            den = at2sb.tile([1, 512], F32, tag="den", bufs=4,
                                             name="den")
                            nc.vector.tensor_copy(den, av[64:65, :])
                            nc.vector.reciprocal_approx_accurate(
                                rec, den, scratch=scr
                            )
                            rd = dram.tile([1, 512], F32, tag="rd", bufs=4,
                                           name="rd")
                            nc.sync.dma_start(out=rd, in_=rec)
                            bc = at2sb.tile([64, 512], F32, tag="bc", bufs=4,
                                           name="bc")
                            nc.sync.dma_start(
                                out=bc,
                                in_=bass.AP(tensor=rd.tensor, offset=rd.offset,
                                            ap=[[0, 64]] + list(rd.ap[1:])),
                            )
                            nc.vector.tensor_mul(
                                osb[g2][64 * h : 64 * h + 64, qs],
                                av[0:64, :],
                                bc,
                            )

            # ================ phase 3: output projection ================
            with (
                nc.named_scope("proj"),
                tc.tile_pool(name="p3sb", bufs=1) as p3sb,
                tc.tile_pool(name="p3ps", bufs=4, space="PSUM") as p3ps,
            ):
                for n in range(8):
                    for c in range(CH):
                        acc = p3ps.tile([128, 512], F32, tag="pp", name="acc")
                        for k in range(4):
                            nc.tensor.matmul(
                                acc,
                                wp_sb[k][:, 128 * n : 128 * n + 128],
                                osb[k][:, 512 * c : 512 * c + 512],
                                start=(k == 0), stop=(k == 3),
                            )
                        yt = p3sb.tile([128, 512], F32, tag="yt", bufs=4,
                                       name="yt")
                        nc.vector.tensor_copy(yt, acc)
                        nc.sync.dma_start(
                            out=yT[128 * n : 128 * n + 128,
                                   512 * c : 512 * c + 512],
                            in_=yt,
                        )
            at2sb_cm.__exit__(None, None, None)
    return nc


_prog = None


def _get_program():
    global _prog
    if _prog is None:
        _prog = build(bacc.Bacc(None))
        _prog.finalize()
    return _prog


def make_in_maps(x, w_qkv, b_qkv, w_proj):
    x = np.ascontiguousarray(np.asarray(x, np.float32))
    w_qkv = np.asarray(w_qkv, np.float32)
    b_qkv = np.asarray(b_qkv, np.float32)
    w_proj = np.asarray(w_proj, np.float32)
    in_maps = []
    for core in range(8):
        b, g = divmod(core, 2)
        gs = slice(DL * g, DL * g + DL)
        gk = slice(D + DL * g, D + DL * g + DL)
        gv = slice(2 * D + DL * g, 2 * D + DL * g + DL)
        in_maps.append({
            "xT": np.ascontiguousarray(x[b].T),
            "wq": np.ascontiguousarray(w_qkv[:, gs]) * np.float32(0.125),
            "wk": np.ascontiguousarray(w_qkv[:, gk]),
            "wv": np.ascontiguousarray(w_qkv[:, gv]),
            "bq": np.ascontiguousarray(b_qkv[gs]) * np.float32(0.125),
            "bk": np.ascontiguousarray(b_qkv[gk]),
            "bv": np.ascontiguousarray(b_qkv[gv]),
            "wp": np.ascontiguousarray(w_proj[DL * g : DL * g + DL, :]),
            "ones8": np.ones((128, 40), np.float32),
        })
    return in_maps


def combine_outputs(results, b_proj):
    b_proj = np.asarray(b_proj, np.float32)
    y = np.empty((B, T, D), np.float32)
    for b in range(B):
        yt = results[2 * b]["yT"] + results[2 * b + 1]["yT"]
        y[b] = yt.T + b_proj
    return y


def kernel(x, w_qkv, b_qkv, w_proj, b_proj, **run_kwargs):
    in_maps = make_in_maps(x, w_qkv, b_qkv, w_proj)
    r = run_bass_kernel_spmd(_get_program(), in_maps,
                             core_ids=list(range(8)), **run_kwargs)
    out = combine_outputs(r.results, b_proj)
    kernel.last_result = r
    return out



# revision 1
# speedup vs baseline: 1.2051x; 1.2051x over previous
"""Causal multi-head attention (B=4, T=2048, D=1024, H=16, HD=64) on 8
Trainium2 NeuronCores.

Sharding: data-parallel over batch (4) x tensor-parallel over heads (2
groups of 8). Each core runs the same Bass program on its own input
slices; the host sums the two tensor-parallel partial projections per
batch and adds b_proj.

Per-core dataflow (feature-major, no on-chip transposes):
  xT [D,T] (host pre-transposed)
  Q^T,K^T = w_{q,k}-stationary matmuls -> [512, T] feature-major
  V       = xT-stationary matmuls      -> [T, 512] token-major (+ones col)
  S^T     = K^T-block-stationary matmuls, 2 heads row-packed in the
            128-deep PE array (contraction = hd = 64)
  P       = exp(S^T) on ScalarE (1/8 scale folded into wq on host),
            causal handled by partial-N matmuls + affine_select on
            diagonal blocks
  O^T,den = V|1-stationary matmul accumulating over tk blocks (the ones
            column yields the softmax denominator in PSUM row 64)
  out     = O^T * (1/den) via reciprocal_approx + DRAM-bounce broadcast
  yT      = w_proj-stationary matmuls -> [D, T] partial (host reduces)

All matmul operands are float32r (streams at 1 cycle/row on the TRN2 PE
vs 4 for float32; ~1.3e-4 relative precision, measured).
"""

import numpy as np

import concourse.bass as bass
import concourse.bacc as bacc
import concourse.mybir as mybir
import concourse.tile as tile
from concourse.bass_utils import run_bass_kernel_spmd

F32 = mybir.dt.float32
F32R = mybir.dt.float32r
AF = mybir.ActivationFunctionType

B, T, D = 4, 2048, 1024
H, HD = 16, 64
NH = 8          # heads per core
DL = NH * HD    # 512 local qkv feature dim
PAIRS = NH // 2
CH = T // 512   # 4 chunks of 512 tokens
KT = T // 128   # 16 tk blocks / token tiles
VW = 65         # V columns per head incl. ones column

USE_F32R = True
MMDT = F32R if USE_F32R else F32


def build(nc: bass.Bass):
    xT = nc.declare_dram_parameter("xT", [D, T], MMDT, isOutput=False)
    wq = nc.declare_dram_parameter("wq", [D, DL], MMDT, isOutput=False)
    wk = nc.declare_dram_parameter("wk", [D, DL], MMDT, isOutput=False)
    wv = nc.declare_dram_parameter("wv", [D, DL], MMDT, isOutput=False)
    bq = nc.declare_dram_parameter("bq", [DL], F32, isOutput=False)
    bk = nc.declare_dram_parameter("bk", [DL], F32, isOutput=False)
    bv = nc.declare_dram_parameter("bv", [DL], F32, isOutput=False)
    wp = nc.declare_dram_parameter("wp", [DL, D], MMDT, isOutput=False)
    ones8 = nc.declare_dram_parameter("ones8", [128, 40], MMDT, isOutput=False)
    yT = nc.declare_dram_parameter("yT", [D, T], F32, isOutput=True)

    with tile.TileContext(nc) as tc:
        with (
            tc.tile_pool(name="persist", bufs=1) as persist,
            tc.tile_pool(name="dram", bufs=4, space="DRAM") as dram,
        ):
            # -------- persistent tiles --------
            qkT = [persist.tile([128, T], MMDT, tag=f"qk{i}", name=f"qk{i}")
                   for i in range(8)]
            v_sb = [persist.tile([128, NH * VW], MMDT, tag=f"v{i}",
                                 name=f"v{i}") for i in range(KT)]
            osb = [persist.tile([128, T], MMDT, tag=f"o{i}", name=f"o{i}")
                   for i in range(4)]
            bq_sb = persist.tile([128, 4], F32, tag="bq", name="bq_sb")
            bk_sb = persist.tile([128, 4], F32, tag="bk", name="bk_sb")
            bv_bc = persist.tile([128, DL], F32, tag="bv", name="bv_bc")
            nc.sync.dma_start(
                out=bq_sb, in_=bq[:].rearrange("(a p) -> p a", p=128)
            )
            nc.sync.dma_start(
                out=bk_sb, in_=bk[:].rearrange("(a p) -> p a", p=128)
            )
            nc.sync.dma_start(
                out=bv_bc,
                in_=bass.AP(tensor=bv[:].tensor, offset=0,
                            ap=[[0, 128], [1, DL]]),
            )

            # ================ phase 1: QKV projections ================
            with (
                nc.named_scope("qkv"),
                tc.tile_pool(name="ph1sb", bufs=1) as ph1sb,
                tc.tile_pool(name="ph1ps", bufs=6, space="PSUM") as ph1ps,
            ):
                wv_sb = [ph1sb.tile([128, DL], MMDT, tag=f"wv{k}",
                                    name=f"wv{k}") for k in range(8)]
                for k in range(8):
                    nc.sync.dma_start(
                        out=wv_sb[k], in_=wv[128 * k : 128 * k + 128, :]
                    )
                for c in range(CH):
                    cs = slice(512 * c, 512 * c + 512)
                    xt = []
                    for k in range(8):
                        t_ = ph1sb.tile([128, 512], MMDT, tag="xt", bufs=12,
                                        name="xt")
                        nc.sync.dma_start(
                            out=t_, in_=xT[128 * k : 128 * k + 128, cs]
                        )
                        xt.append(t_)
                    # V token-major (first: attention waits on all of V)
                    for t4 in range(4):
                        tt = 4 * c + t4
                        acc = ph1ps.tile([128, 512], F32, tag="ps", name="acc")
                        for k in range(8):
                            nc.tensor.matmul(
                                acc,
                                xt[k][:, 128 * t4 : 128 * t4 + 128],
                                wv_sb[k],
                                start=(k == 0),
                                stop=(k == 7),
                            )
                        # ones in col 64 of each 65-wide head block
                        nc.sync.dma_start(
                            out=v_sb[tt].rearrange("p (h c) -> p h c", c=VW)
                            [:, :, HD],
                            in_=ones8[:, 0:NH],
                        )
                        for h in range(NH):
                            nc.vector.tensor_add(
                                v_sb[tt][:, VW * h : VW * h + HD],
                                acc[:, HD * h : HD * h + HD],
                                bv_bc[:, HD * h : HD * h + HD],
                            )
                    # Q^T then K^T feature-major
                    for w_in, b_sb, obase in ((wq, bq_sb, 0), (wk, bk_sb, 4)):
                        wts = []
                        for k in range(8):
                            wt = ph1sb.tile([128, DL], MMDT, tag="w",
                                            bufs=8, name="wt")
                            nc.sync.dma_start(
                                out=wt, in_=w_in[128 * k : 128 * k + 128, :]
                            )
                            wts.append(wt)
                        for n in range(4):
                            acc = ph1ps.tile([128, 512], F32, tag="ps",
                                             name="acc")
                            for k in range(8):
                                nc.tensor.matmul(
                                    acc,
                                    wts[k][:, 128 * n : 128 * n + 128],
                                    xt[k],
                                    start=(k == 0), stop=(k == 7),
                                )
                            nc.vector.tensor_scalar_add(
                                out=qkT[obase + n][:, cs],
                                in0=acc,
                                scalar1=b_sb[:, n : n + 1],
                            )

            # prefetch proj weights (overlaps attention)
            at2sb_cm = tc.tile_pool(name="at2sb", bufs=1)
            at2sb = at2sb_cm.__enter__()
            wp_sb = [at2sb.tile([128, D], MMDT, tag=f"wp{k}",
                                name=f"wp{k}") for k in range(4)]
            for k in range(4):
                nc.sync.dma_start(
                    out=wp_sb[k], in_=wp[128 * k : 128 * k + 128, :]
                )

            # ================ phase 2: attention ================
            with (
                nc.named_scope("attn"),
                tc.tile_pool(name="atps", bufs=1, space="PSUM") as atps,
            ):
                for g2 in range(PAIRS):
                    qt, kt = qkT[g2], qkT[4 + g2]
                    ha, hb = 2 * g2, 2 * g2 + 1
                    for c in range(CH):
                        qs = slice(512 * c, 512 * c + 512)
                        av_a = atps.tile([VW, 512], F32, tag="av", bufs=4,
                                         name="av_a")
                        av_b = atps.tile([VW, 512], F32, tag="av", bufs=4,
                                         name="av_b")
                        nb = 4 * (c + 1)
                        for b in range(nb):
                            diag = (b // 4 == c)
                            off = 128 * (b - 4 * c) if diag else 0
                            bs = slice(128 * b, 128 * b + 128)
                            strip = atps.tile([128, 1024], F32, tag="strip",
                                              bufs=2, name="strip")
                            et = at2sb.tile([128, 1024], MMDT, tag="exp",
                                            bufs=4, name="et")
                            nc.tensor.matmul(
                                strip[:, off:512],
                                kt[0:64, bs],
                                qt[0:64, 512 * c + off : 512 * c + 512],
                                start=True, stop=True,
                            )
                            nc.tensor.matmul(
                                strip[:, 512 + off : 1024],
                                kt[64:128, bs],
                                qt[64:128, 512 * c + off : 512 * c + 512],
                                start=True, stop=True,
                            )
                            if off == 0:
                                nc.scalar.activation(
                                    et[:, 0:1024], strip[:, 0:1024], AF.Exp
                                )
                            else:
                                # one instr over both heads' valid regions:
                                # cols [off,512) and [512+off,1024)
                                w_ = 512 - off
                                src_ap = bass.AP(
                                    tensor=strip.tensor,
                                    offset=strip.offset + off,
                                    ap=[list(strip.ap[0]), [512, 2], [1, w_]],
                                )
                                dst_ap = bass.AP(
                                    tensor=et.tensor,
                                    offset=et.offset + off,
                                    ap=[list(et.ap[0]), [512, 2], [1, w_]],
                                )
                                nc.scalar.activation(dst_ap, src_ap, AF.Exp)
                            if diag:
                                for bcol in (off, 512 + off):
                                    nc.gpsimd.affine_select(
                                        out=et[:, bcol : bcol + 128],
                                        in_=et[:, bcol : bcol + 128],
                                        compare_op=mybir.AluOpType.is_ge,
                                        fill=0.0,
                                        base=0,
                                        pattern=[[1, 128]],
                                        channel_multiplier=-1,
                                    )
                            nc.tensor.matmul(
                                av_a[:, off:512],
                                v_sb[b][:, VW * ha : VW * ha + VW],
                                et[:, off:512],
                                start=(b == 0), stop=(b == nb - 1),
                            )
                            nc.tensor.matmul(
                                av_b[:, off:512],
                                v_sb[b][:, VW * hb : VW * hb + VW],
                                et[:, 512 + off : 1024],
                                start=(b == 0), stop=(b == nb - 1),
                            )
                        for h, av in ((0, av_a), (1, av_b)):
                            rec = at2sb.tile([1, 512], F32, tag="rec", bufs=4,
                                             name="rec")
                            scr = at2sb.tile([1, 512], F32, tag="scr", bufs=4,
                                             name="scr")
                            den = at2sb.tile([1, 512], F32, tag="den", bufs=4,
                                             name="den")
                            nc.vector.tensor_copy(den, av[64:65, :])
                            nc.vector.reciprocal_approx_accurate(
                                rec, den, scratch=scr
                            )
                            rd = dram.tile([1, 512], F32, tag="rd", bufs=4,
                                           name="rd")
                            nc.sync.dma_start(out=rd, in_=rec)
                            bc = at2sb.tile([64, 512], F32, tag="bc", bufs=4,
                                           name="bc")
                            nc.sync.dma_start(
                                out=bc,
                                in_=bass.AP(tensor=rd.tensor, offset=rd.offset,
                                            ap=[[0, 64]] + list(rd.ap[1:])),
                            )
                            nc.vector.tensor_mul(
                                osb[g2][64 * h : 64 * h + 64, qs],
                                av[0:64, :],
                                bc,
                            )

            # ================ phase 3: output projection ================
            with (
                nc.named_scope("proj"),
                tc.tile_pool(name="p3sb", bufs=1) as p3sb,
                tc.tile_pool(name="p3ps", bufs=4, space="PSUM") as p3ps,
            ):
                for n in range(8):
                    for c in range(CH):
                        acc = p3ps.tile([128, 512], F32, tag="pp", name="acc")
                        for k in range(4):
                            nc.tensor.matmul(
                                acc,
                                wp_sb[k][:, 128 * n : 128 * n + 128],
                                osb[k][:, 512 * c : 512 * c + 512],
                                start=(k == 0), stop=(k == 3),
                            )
                        yt = p3sb.tile([128, 512], F32, tag="yt", bufs=4,
                                       name="yt")
                        nc.vector.tensor_copy(yt, acc)
                        nc.sync.dma_start(
                            out=yT[128 * n : 128 * n + 128,
                                   512 * c : 512 * c + 512],
                            in_=yt,
                        )
            at2sb_cm.__exit__(None, None, None)
    return nc


_prog = None


def _get_program():
    global _prog
    if _prog is None:
        _prog = build(bacc.Bacc(None))
        _prog.finalize()
    return _prog


def make_in_maps(x, w_qkv, b_qkv, w_proj):
    x = np.ascontiguousarray(np.asarray(x, np.float32))
    w_qkv = np.asarray(w_qkv, np.float32)
    b_qkv = np.asarray(b_qkv, np.float32)
    w_proj = np.asarray(w_proj, np.float32)
    in_maps = []
    for core in range(8):
        b, g = divmod(core, 2)
        gs = slice(DL * g, DL * g + DL)
        gk = slice(D + DL * g, D + DL * g + DL)
        gv = slice(2 * D + DL * g, 2 * D + DL * g + DL)
        in_maps.append({
            "xT": np.ascontiguousarray(x[b].T),
            "wq": np.ascontiguousarray(w_qkv[:, gs]) * np.float32(0.125),
            "wk": np.ascontiguousarray(w_qkv[:, gk]),
            "wv": np.ascontiguousarray(w_qkv[:, gv]),
            "bq": np.ascontiguousarray(b_qkv[gs]) * np.float32(0.125),
            "bk": np.ascontiguousarray(b_qkv[gk]),
            "bv": np.ascontiguousarray(b_qkv[gv]),
            "wp": np.ascontiguousarray(w_proj[DL * g : DL * g + DL, :]),
            "ones8": np.ones((128, 40), np.float32),
        })
    return in_maps


def combine_outputs(results, b_proj):
    b_proj = np.asarray(b_proj, np.float32)
    y = np.empty((B, T, D), np.float32)
    for b in range(B):
        yt = results[2 * b]["yT"] + results[2 * b + 1]["yT"]
        y[b] = yt.T + b_proj
    return y


def kernel(x, w_qkv, b_qkv, w_proj, b_proj, **run_kwargs):
    in_maps = make_in_maps(x, w_qkv, b_qkv, w_proj)
    r = run_bass_kernel_spmd(_get_program(), in_maps,
                             core_ids=list(range(8)), **run_kwargs)
    out = combine_outputs(r.results, b_proj)
    kernel.last_result = r
    return out

